# revision 2
# baseline (speedup 1.0000x reference)
"""Trainium2 Bass kernel v3 for nn_CompositionalNetwork (ragged_sequence).

Per-token embedding concat (word[200] ++ tag[20]) -> per-chunk-length Linear
over chunks of 1..4 consecutive tokens -> scatter rows by pos.

Gather: InstDMAGatherAnt (vectorized Q7 descriptor generation) in
NON-transpose mode, round-robin over 4 SWDGE queues (multi-queue transpose
mode corrupts via the shared xbar; non-transpose is multi-queue-safe and
4 queues quadruple descriptor-generation throughput). dma_gather indices are
int16, so each core's 51200-token stream is split into two 25600-slot
windows with per-window host-compacted word tables (distinct rows in
first-use order -> near-sequential HBM access).

Compute: per 128-chunk tile, PE transposes flip [chunks, feat] into
[feat, chunks] (PSUM), DVE copies stage them to SBUF, then per position j
two matmuls contract word features, plus one matmul for the host-pretransposed
tag+bias slab:
  y[chunk,:] = sum_j xw_j[0:128] @ W0[j] + sum_j xw_j[128:200] @ W1[j]
             + slab[0:20k+1] @ Wslab_k

Sharding: data-parallel by output row range; per-core output is a contiguous
[20000, 200] block written with a strided affine DMA (row = 4*i + k-1), with
an indirect-scatter fallback if pos is not affine.
"""
import numpy as np
import ml_dtypes

bf16 = ml_dtypes.bfloat16

VOCAB = 128000
TAGS = 64
WD = 200
TD = 20
E = WD + TD
CD = 200
K = 4
C = 40000
S = 400000
NCH = K * C

NCORES = 8
P = 128
RW = 256            # padded compact-table row: 256 bf16 = 512 B
CG = 5120           # padded chunks per k-group per core
CPG = C // NCORES   # real chunks per group per core (5000)
OUTR = 4 * CG       # local out rows incl pad targets (20480)
NBT = 8             # tiles per block
NB = CG // (NBT * P)  # blocks per group (5)
NBLK = NBT * P      # chunks per block (1024)
NSLOT = CG * (1 + 2 + 3 + 4)  # 51200 gather slots per core
WSLOT = NSLOT // 2  # slots per index window (25600)
NCT = 2 * WSLOT     # compact table rows (2 windows of <=25600 distinct)
SLABP = 96          # slab partition dim (>= 20*K+1 = 81)
NQ = 4              # SWDGE queues

_CACHE = {}


def _build_kernel(affine):
    from concourse import bacc
    import concourse.tile as tile
    from concourse import mybir
    import concourse.bass as bass
    from concourse.bass import IndirectOffsetOnAxis
    from concourse.masks import make_identity

    nc = bacc.Bacc(None, num_swdge_queues=NQ)

    ctab = nc.dram_tensor("ctab", [NCT, RW], mybir.dt.bfloat16, kind="ExternalInput")
    idx_d = nc.dram_tensor("idx", [P, NSLOT // 16], mybir.dt.int16, kind="ExternalInput")
    slab_d = nc.dram_tensor("slab", [SLABP, K * CG], mybir.dt.bfloat16, kind="ExternalInput")
    w0_d = nc.dram_tensor("w0", [10, P, CD], mybir.dt.bfloat16, kind="ExternalInput")
    w1_d = nc.dram_tensor("w1", [10, 72, CD], mybir.dt.bfloat16, kind="ExternalInput")
    ws_d = nc.dram_tensor("ws", [K, SLABP, CD], mybir.dt.bfloat16, kind="ExternalInput")
    pos_d = nc.dram_tensor("pos", [P, (CG // P) * K], mybir.dt.int32, kind="ExternalInput")
    out = nc.dram_tensor("out", [OUTR, CD], mybir.dt.float32, kind="ExternalOutput")

    with tile.TileContext(nc) as tc:
        with (
            tc.tile_pool(name="singles", bufs=1) as singles,
            tc.tile_pool(name="xp", bufs=12) as xp,
            tc.tile_pool(name="slp", bufs=3) as slp,
            tc.tile_pool(name="xtp", bufs=6) as xtp,
            tc.tile_pool(name="ysp", bufs=3) as ysp,
            tc.tile_pool(name="tpp", bufs=4, space="PSUM") as tpp,
            tc.tile_pool(name="ypp", bufs=4, space="PSUM") as ypp,
        ):
            ident = singles.tile([P, P], mybir.dt.bfloat16)
            make_identity(nc, ident[:])
            sidx = singles.tile([P, NSLOT // 16], mybir.dt.int16)
            nc.sync.dma_start(out=sidx[:], in_=idx_d[:])
            w0 = singles.tile([P, 10, CD], mybir.dt.bfloat16)
            nc.sync.dma_start(out=w0[:], in_=w0_d[:].rearrange("q f c -> f q c"))
            w1 = singles.tile([72, 10, CD], mybir.dt.bfloat16)
            nc.sync.dma_start(out=w1[:], in_=w1_d[:].rearrange("q f c -> f q c"))
            ws = singles.tile([SLABP, K, CD], mybir.dt.bfloat16)
            nc.sync.dma_start(out=ws[:], in_=ws_d[:].rearrange("k f c -> f k c"))
            spos = None
            if not affine:
                spos = singles.tile([P, (CG // P) * K], mybir.dt.int32)
                nc.sync.dma_start(out=spos[:], in_=pos_d[:])

            gq = [0]
            slot_base = 0
            for k in range(1, K + 1):
                q0 = (k - 1) * k // 2
                for b in range(NB):
                    sl = slp.tile([SLABP, NBLK], mybir.dt.bfloat16, tag="sl")
                    c0 = (k - 1) * CG + b * NBLK
                    nc.sync.dma_start(out=sl[:], in_=slab_d[:, c0:c0 + NBLK])
                    xts = []
                    for j in range(k):
                        xt = xp.tile([P, NBT, RW], mybir.dt.bfloat16, tag="x")
                        s0 = slot_base + (b * k + j) * NBLK
                        src = ctab[0:WSLOT] if s0 < WSLOT else ctab[WSLOT:NCT]
                        nc.gpsimd.dma_gather(
                            xt[:], src, sidx[:, s0 // 16:(s0 + NBLK) // 16],
                            NBLK, NBLK, RW, transpose=False,
                            queue_num=gq[0] % NQ, single_packet=False,
                        )
                        gq[0] += 1
                        xts.append(xt)
                    ystage = ysp.tile([P, NBT, CD], mybir.dt.float32)
                    for t in range(NBT):
                        y = ypp.tile([P, CD], mybir.dt.float32)
                        cs = t * P
                        for j in range(k):
                            tp = tpp.tile([P, 2 * P], mybir.dt.bfloat16)
                            nc.tensor.transpose(tp[0:P, 0:P], xts[j][:, t, 0:128], ident[:])
                            nc.tensor.transpose(tp[0:72, P:2 * P], xts[j][:, t, 128:200], ident[:])
                            xT = xtp.tile([P, 2 * P], mybir.dt.bfloat16, tag="xT")
                            nc.vector.tensor_copy(xT[:, 0:P], tp[:, 0:P])
                            nc.vector.tensor_copy(xT[0:72, P:2 * P], tp[0:72, P:2 * P])
                            nc.tensor.matmul(
                                y[:], lhsT=xT[:, 0:P], rhs=w0[:, q0 + j, :],
                                start=(j == 0), stop=False,
                            )
                            nc.tensor.matmul(
                                y[:], lhsT=xT[0:72, P:2 * P], rhs=w1[0:72, q0 + j, :],
                                start=False, stop=False,
                            )
                        nc.tensor.matmul(
                            y[:], lhsT=sl[0:20 * k + 1, cs:cs + P],
                            rhs=ws[0:20 * k + 1, k - 1, :],
                            start=False, stop=True,
                        )
                        nc.vector.tensor_copy(ystage[:, t, :], y[:])
                    if affine:
                        # out row = 4*(b*NBLK + t*128 + p) + (k-1)
                        dst = bass.AP(
                            tensor=out[:].tensor,
                            offset=(4 * NBLK * b + (k - 1)) * CD,
                            ap=[[4 * CD, P], [4 * P * CD, NBT], [1, CD]],
                        )
                        nc.sync.dma_start(out=dst, in_=ystage[:, :, :])
                    else:
                        for t in range(NBT):
                            tt = b * NBT + t
                            nc.gpsimd.indirect_dma_start(
                                out=out[:],
                                out_offset=IndirectOffsetOnAxis(
                                    ap=spos[:, (k - 1) * (CG // P) + tt:(k - 1) * (CG // P) + tt + 1],
                                    axis=0,
                                ),
                                in_=ystage[:, t, :],
                                in_offset=None,
                            )
                slot_base += k * CG
    nc.compile()
    return nc


def _prep(inputs):
    """Host-side shard + pack. Returns (affine, in_maps)."""
    tok = np.asarray(inputs["token_indices"]).astype(np.int64)
    tag = np.asarray(inputs["tag_indices"]).astype(np.int64)
    word_table = np.asarray(inputs["word_table"], dtype=np.float32)
    tag_table = np.asarray(inputs["tag_table"], dtype=np.float32)

    wtab_bf = word_table.astype(bf16)             # [V, 200]
    tagemb = tag_table.astype(bf16)               # [TAGS, 20]

    # packed weights (shared by all cores)
    w0 = np.zeros((10, P, CD), dtype=np.float32)
    w1 = np.zeros((10, 72, CD), dtype=np.float32)
    ws = np.zeros((K, SLABP, CD), dtype=np.float32)
    for k in range(1, K + 1):
        Wk = np.asarray(inputs[f"W{k}"], dtype=np.float32)
        bk = np.asarray(inputs[f"b{k}"], dtype=np.float32)
        q0 = (k - 1) * k // 2
        for j in range(k):
            off = j * E
            w0[q0 + j] = Wk[:, off:off + 128].T
            w1[q0 + j] = Wk[:, off + 128:off + 200].T
            ws[k - 1, 20 * j:20 * j + 20] = Wk[:, off + 200:off + 220].T
        ws[k - 1, 20 * k] = bk
    w0 = w0.astype(bf16)
    w1 = w1.astype(bf16)
    ws = ws.astype(bf16)

    affine = True
    shards = []  # per core: dict k -> (chunk_ids[CG], local_pos[CG], n)
    for c in range(NCORES):
        lo, hi = c * (NCH // NCORES), (c + 1) * (NCH // NCORES)
        per_k = {}
        for k in range(1, K + 1):
            pos = np.asarray(inputs[f"pos{k}"]).astype(np.int64)
            sel = np.nonzero((pos >= lo) & (pos < hi))[0]
            lp = pos[sel] - lo
            order = np.argsort(lp, kind="stable")
            sel = sel[order]
            lp = lp[order]
            n = len(sel)
            if n > CG:
                raise ValueError("shard overflow; unbalanced pos distribution")
            if n != CPG or not np.array_equal(lp, 4 * np.arange(n) + (k - 1)):
                affine = False
            per_k[k] = (sel, lp, n)
        shards.append(per_k)

    in_maps = []
    for c in range(NCORES):
        # ---- gather slot stream: vocab id per slot, in op order ----
        vids = np.zeros(NSLOT, dtype=np.int64)
        slot_base = 0
        for k in range(1, K + 1):
            starts = np.asarray(inputs[f"starts{k}"]).astype(np.int64)
            sel, lp, n = shards[c][k]
            st = np.zeros(CG, dtype=np.int64)
            st[:n] = starts[sel]
            for b in range(NB):
                for j in range(k):
                    s0 = slot_base + (b * k + j) * NBLK
                    cc = st[b * NBLK:(b + 1) * NBLK] + j
                    vids[s0:s0 + NBLK] = tok[np.clip(cc, 0, S - 1)]
            slot_base += k * CG

        # ---- per-window dedup -> compact table + int16 indices ----
        idx16 = np.zeros(NSLOT, dtype=np.int16)
        ctab = np.zeros((NCT, RW), dtype=bf16)
        for w in range(2):
            sl = slice(w * WSLOT, (w + 1) * WSLOT)
            uniq, inv = np.unique(vids[sl], return_inverse=True)
            first_use = np.full(len(uniq), NSLOT, dtype=np.int64)
            np.minimum.at(first_use, inv, np.arange(WSLOT))
            order = np.argsort(first_use, kind="stable")
            rank = np.empty_like(order)
            rank[order] = np.arange(len(uniq))
            nw = len(uniq)
            assert nw <= WSLOT
            idx16[sl] = rank[inv].astype(np.int16)
            ctab[w * WSLOT:w * WSLOT + nw, 0:WD] = wtab_bf[uniq[order]]

        # idx packed layout: slot i -> [16*g + i%16, i//16] replicated g=0..7
        idxp = np.tile(idx16.reshape(NSLOT // 16, 16).T, (8, 1))

        # ---- slab: [96, K*CG] tag embeddings + ones row ----
        slab = np.zeros((SLABP, K * CG), dtype=bf16)
        posarr = np.zeros((P, (CG // P) * K), dtype=np.int32)
        for k in range(1, K + 1):
            starts = np.asarray(inputs[f"starts{k}"]).astype(np.int64)
            sel, lp, n = shards[c][k]
            st = np.zeros(CG, dtype=np.int64)
            st[:n] = starts[sel]
            col0 = (k - 1) * CG
            for j in range(k):
                tg = tag[np.clip(st + j, 0, S - 1)]
                tv = tagemb[tg]                     # [CG, 20]
                tv[n:] = 0
                slab[20 * j:20 * j + 20, col0:col0 + CG] = tv.T
            onesr = np.zeros(CG, dtype=bf16)
            onesr[:n] = 1.0
            slab[20 * k, col0:col0 + CG] = onesr
            lpp = np.full(CG, OUTR - P, dtype=np.int64)
            lpp[:n] = lp
            posarr[:, (k - 1) * (CG // P):k * (CG // P)] = lpp.reshape(CG // P, P).T
        in_maps.append(dict(ctab=ctab, idx=idxp, slab=slab, w0=w0, w1=w1, ws=ws,
                            pos=posarr))
    return affine, in_maps


def kernel(**inputs) -> np.ndarray:
    from concourse.bass_utils import run_bass_kernel_spmd

    affine, in_maps = _prep(inputs)

    key = ("nc", affine)
    if key not in _CACHE:
        _CACHE[key] = _build_kernel(affine)
    nc = _CACHE[key]

    res = run_bass_kernel_spmd(nc, in_maps, list(range(NCORES)))

    per = NCH // NCORES
    blocks = [np.asarray(res.results[c]["out"])[:per] for c in range(NCORES)]
    return np.concatenate(blocks, axis=0).astype(np.float32)


# revision 3
# speedup vs baseline: 2.4763x; 2.4763x over previous
"""Trainium2 Bass kernel v5 for nn_CompositionalNetwork (ragged_sequence).

Per-token embedding concat (word[200] ++ tag[20]) -> per-chunk-length Linear
over chunks of 1..4 consecutive tokens -> scatter rows by pos.

Gather: InstDMAGatherAnt (vectorized Q7 descriptor generation) in
NON-transpose mode, round-robin over 4 SWDGE queues (multi-queue transpose
mode corrupts via the shared xbar; non-transpose is multi-queue-safe and
4 queues quadruple descriptor-generation throughput). dma_gather indices are
int16, so each core's 51200-token stream is split into two 25600-slot
windows with per-window host-compacted word tables (distinct rows in
first-use order -> near-sequential HBM access).

Compute: per 128-chunk tile, PE transposes flip [chunks, feat] into
[feat, chunks] (PSUM), DVE copies stage them to SBUF, then per position j
two matmuls contract word features, plus one matmul for the host-pretransposed
tag+bias slab:
  y[chunk,:] = sum_j xw_j[0:128] @ W0[j] + sum_j xw_j[128:200] @ W1[j]
             + slab[0:20k+1] @ Wslab_k

Sharding: data-parallel by output row range; per-core output is a contiguous
[20000, 200] block written with a strided affine DMA (row = 4*i + k-1), with
an indirect-scatter fallback if pos is not affine.
"""
import numpy as np
import ml_dtypes

bf16 = ml_dtypes.bfloat16

VOCAB = 128000
TAGS = 64
WD = 200
TD = 20
E = WD + TD
CD = 200
K = 4
C = 40000
S = 400000
NCH = K * C

NCORES = 8
P = 128
RW = 256            # padded compact-table row: 256 bf16 = 512 B
CG = 5120           # padded chunks per k-group per core
CPG = C // NCORES   # real chunks per group per core (5000)
OUTR = 4 * CG       # local out rows incl pad targets (20480)
NBT = 16            # max tiles per block
NBLK = NBT * P      # max chunks per block (2048)


def _blocks_for_group():
    """CG=5120 chunks per group: blocks of 2048, 2048, 1024."""
    blks = []
    off = 0
    while off < CG:
        n = min(NBLK, CG - off)
        blks.append((off, n))
        off += n
    return blks
NSLOT = CG * (1 + 2 + 3 + 4)  # 51200 gather slots per core
WSLOT = NSLOT // 2  # slots per index window (25600)
NCT = 2 * WSLOT     # compact table rows (2 windows of <=25600 distinct)
SLABP = 96          # slab partition dim (>= 20*K+1 = 81)
NQ = 4              # SWDGE queues

_CACHE = {}


def _build_kernel(affine):
    from concourse import bacc
    import concourse.tile as tile
    from concourse import mybir
    import concourse.bass as bass
    from concourse.bass import IndirectOffsetOnAxis
    from concourse.masks import make_identity

    nc = bacc.Bacc(None, num_swdge_queues=NQ)

    ctab = nc.dram_tensor("ctab", [NCT, RW], mybir.dt.bfloat16, kind="ExternalInput")
    idx_d = nc.dram_tensor("idx", [P, NSLOT // 16], mybir.dt.int16, kind="ExternalInput")
    slab_d = nc.dram_tensor("slab", [SLABP, K * CG], mybir.dt.bfloat16, kind="ExternalInput")
    w0_d = nc.dram_tensor("w0", [10, P, CD], mybir.dt.bfloat16, kind="ExternalInput")
    w1_d = nc.dram_tensor("w1", [10, 72, CD], mybir.dt.bfloat16, kind="ExternalInput")
    ws_d = nc.dram_tensor("ws", [K, SLABP, CD], mybir.dt.bfloat16, kind="ExternalInput")
    pos_d = nc.dram_tensor("pos", [P, (CG // P) * K], mybir.dt.int32, kind="ExternalInput")
    out = nc.dram_tensor("out", [OUTR, CD], mybir.dt.float32, kind="ExternalOutput")

    with tile.TileContext(nc) as tc:
        with (
            tc.tile_pool(name="singles", bufs=1) as singles,
            tc.tile_pool(name="xp", bufs=8) as xp,
            tc.tile_pool(name="slp", bufs=3) as slp,
            tc.tile_pool(name="xtp", bufs=6) as xtp,
            tc.tile_pool(name="ysp", bufs=3) as ysp,
            tc.tile_pool(name="tpp", bufs=4, space="PSUM") as tpp,
            tc.tile_pool(name="ypp", bufs=4, space="PSUM") as ypp,
        ):
            ident = singles.tile([P, P], mybir.dt.bfloat16)
            make_identity(nc, ident[:])
            sidx = singles.tile([P, NSLOT // 16], mybir.dt.int16)
            nc.sync.dma_start(out=sidx[:], in_=idx_d[:])
            w0 = singles.tile([P, 10, CD], mybir.dt.bfloat16)
            nc.sync.dma_start(out=w0[:], in_=w0_d[:].rearrange("q f c -> f q c"))
            w1 = singles.tile([72, 10, CD], mybir.dt.bfloat16)
            nc.sync.dma_start(out=w1[:], in_=w1_d[:].rearrange("q f c -> f q c"))
            ws = singles.tile([SLABP, K, CD], mybir.dt.bfloat16)
            nc.sync.dma_start(out=ws[:], in_=ws_d[:].rearrange("k f c -> f k c"))
            spos = None
            if not affine:
                spos = singles.tile([P, (CG // P) * K], mybir.dt.int32)
                nc.sync.dma_start(out=spos[:], in_=pos_d[:])

            gq = [0]
            slot_base = 0
            for k in range(1, K + 1):
                q0 = (k - 1) * k // 2
                for (boff, bn) in _blocks_for_group():
                    nbt = bn // P
                    sl = slp.tile([SLABP, bn], mybir.dt.bfloat16, tag="sl")
                    c0 = (k - 1) * CG + boff
                    nc.sync.dma_start(out=sl[:], in_=slab_d[:, c0:c0 + bn])
                    xts = []
                    for j in range(k):
                        xt = xp.tile([P, nbt, RW], mybir.dt.bfloat16, tag="x")
                        s0 = slot_base + boff * k + j * bn
                        assert (s0 < WSLOT) == (s0 + bn <= WSLOT)
                        src = ctab[0:WSLOT] if s0 < WSLOT else ctab[WSLOT:NCT]
                        nc.gpsimd.dma_gather(
                            xt[:], src, sidx[:, s0 // 16:(s0 + bn) // 16],
                            bn, bn, RW, transpose=False,
                            queue_num=gq[0] % NQ, single_packet=False,
                        )
                        gq[0] += 1
                        xts.append(xt)
                    ystage = ysp.tile([P, nbt, CD], mybir.dt.float32)
                    for t in range(nbt):
                        y = ypp.tile([P, CD], mybir.dt.float32)
                        cs = t * P
                        for j in range(k):
                            tp = tpp.tile([P, 2 * P], mybir.dt.bfloat16)
                            nc.tensor.transpose(tp[0:P, 0:P], xts[j][:, t, 0:128], ident[:])
                            nc.tensor.transpose(tp[0:72, P:2 * P], xts[j][:, t, 128:200], ident[:])
                            xT = xtp.tile([P, 2 * P], mybir.dt.bfloat16, tag="xT")
                            nc.vector.tensor_copy(xT[:, 0:P], tp[:, 0:P])
                            nc.vector.tensor_copy(xT[0:72, P:2 * P], tp[0:72, P:2 * P])
                            nc.tensor.matmul(
                                y[:], lhsT=xT[:, 0:P], rhs=w0[:, q0 + j, :],
                                start=(j == 0), stop=False,
                            )
                            nc.tensor.matmul(
                                y[:], lhsT=xT[0:72, P:2 * P], rhs=w1[0:72, q0 + j, :],
                                start=False, stop=False,
                            )
                        nc.tensor.matmul(
                            y[:], lhsT=sl[0:20 * k + 1, cs:cs + P],
                            rhs=ws[0:20 * k + 1, k - 1, :],
                            start=False, stop=True,
                        )
                        nc.vector.tensor_copy(ystage[:, t, :], y[:])
                    if affine:
                        # out row = 4*(boff + t*128 + p) + (k-1)
                        dst = bass.AP(
                            tensor=out[:].tensor,
                            offset=(4 * boff + (k - 1)) * CD,
                            ap=[[4 * CD, P], [4 * P * CD, nbt], [1, CD]],
                        )
                        nc.sync.dma_start(out=dst, in_=ystage[:, :, :])
                    else:
                        for t in range(nbt):
                            tt = boff // P + t
                            nc.gpsimd.indirect_dma_start(
                                out=out[:],
                                out_offset=IndirectOffsetOnAxis(
                                    ap=spos[:, (k - 1) * (CG // P) + tt:(k - 1) * (CG // P) + tt + 1],
                                    axis=0,
                                ),
                                in_=ystage[:, t, :],
                                in_offset=None,
                            )
                slot_base += k * CG
    nc.compile()
    return nc


def _prep(inputs):
    """Host-side shard + pack. Returns (affine, in_maps)."""
    tok = np.asarray(inputs["token_indices"]).astype(np.int64)
    tag = np.asarray(inputs["tag_indices"]).astype(np.int64)
    word_table = np.asarray(inputs["word_table"], dtype=np.float32)
    tag_table = np.asarray(inputs["tag_table"], dtype=np.float32)

    wtab_bf = word_table.astype(bf16)             # [V, 200]
    tagemb = tag_table.astype(bf16)               # [TAGS, 20]

    # packed weights (shared by all cores)
    w0 = np.zeros((10, P, CD), dtype=np.float32)
    w1 = np.zeros((10, 72, CD), dtype=np.float32)
    ws = np.zeros((K, SLABP, CD), dtype=np.float32)
    for k in range(1, K + 1):
        Wk = np.asarray(inputs[f"W{k}"], dtype=np.float32)
        bk = np.asarray(inputs[f"b{k}"], dtype=np.float32)
        q0 = (k - 1) * k // 2
        for j in range(k):
            off = j * E
            w0[q0 + j] = Wk[:, off:off + 128].T
            w1[q0 + j] = Wk[:, off + 128:off + 200].T
            ws[k - 1, 20 * j:20 * j + 20] = Wk[:, off + 200:off + 220].T
        ws[k - 1, 20 * k] = bk
    w0 = w0.astype(bf16)
    w1 = w1.astype(bf16)
    ws = ws.astype(bf16)

    affine = True
    shards = []  # per core: dict k -> (chunk_ids[CG], local_pos[CG], n)
    for c in range(NCORES):
        lo, hi = c * (NCH // NCORES), (c + 1) * (NCH // NCORES)
        per_k = {}
        for k in range(1, K + 1):
            pos = np.asarray(inputs[f"pos{k}"]).astype(np.int64)
            sel = np.nonzero((pos >= lo) & (pos < hi))[0]
            lp = pos[sel] - lo
            order = np.argsort(lp, kind="stable")
            sel = sel[order]
            lp = lp[order]
            n = len(sel)
            if n > CG:
                raise ValueError("shard overflow; unbalanced pos distribution")
            if n != CPG or not np.array_equal(lp, 4 * np.arange(n) + (k - 1)):
                affine = False
            per_k[k] = (sel, lp, n)
        shards.append(per_k)

    in_maps = []
    for c in range(NCORES):
        # ---- gather slot stream: vocab id per slot, in op order ----
        vids = np.zeros(NSLOT, dtype=np.int64)
        slot_base = 0
        for k in range(1, K + 1):
            starts = np.asarray(inputs[f"starts{k}"]).astype(np.int64)
            sel, lp, n = shards[c][k]
            st = np.zeros(CG, dtype=np.int64)
            st[:n] = starts[sel]
            for (boff, bn) in _blocks_for_group():
                for j in range(k):
                    s0 = slot_base + boff * k + j * bn
                    cc = st[boff:boff + bn] + j
                    vids[s0:s0 + bn] = tok[np.clip(cc, 0, S - 1)]
            slot_base += k * CG

        # ---- per-window dedup -> compact table + int16 indices ----
        idx16 = np.zeros(NSLOT, dtype=np.int16)
        ctab = np.zeros((NCT, RW), dtype=bf16)
        for w in range(2):
            sl = slice(w * WSLOT, (w + 1) * WSLOT)
            uniq, inv = np.unique(vids[sl], return_inverse=True)
            first_use = np.full(len(uniq), NSLOT, dtype=np.int64)
            np.minimum.at(first_use, inv, np.arange(WSLOT))
            order = np.argsort(first_use, kind="stable")
            rank = np.empty_like(order)
            rank[order] = np.arange(len(uniq))
            nw = len(uniq)
            assert nw <= WSLOT
            idx16[sl] = rank[inv].astype(np.int16)
            ctab[w * WSLOT:w * WSLOT + nw, 0:WD] = wtab_bf[uniq[order]]

        # idx packed layout: slot i -> [16*g + i%16, i//16] replicated g=0..7
        idxp = np.tile(idx16.reshape(NSLOT // 16, 16).T, (8, 1))

        # ---- slab: [96, K*CG] tag embeddings + ones row ----
        slab = np.zeros((SLABP, K * CG), dtype=bf16)
        posarr = np.zeros((P, (CG // P) * K), dtype=np.int32)
        for k in range(1, K + 1):
            starts = np.asarray(inputs[f"starts{k}"]).astype(np.int64)
            sel, lp, n = shards[c][k]
            st = np.zeros(CG, dtype=np.int64)
            st[:n] = starts[sel]
            col0 = (k - 1) * CG
            for j in range(k):
                tg = tag[np.clip(st + j, 0, S - 1)]
                tv = tagemb[tg]                     # [CG, 20]
                tv[n:] = 0
                slab[20 * j:20 * j + 20, col0:col0 + CG] = tv.T
            onesr = np.zeros(CG, dtype=bf16)
            onesr[:n] = 1.0
            slab[20 * k, col0:col0 + CG] = onesr
            lpp = np.full(CG, OUTR - P, dtype=np.int64)
            lpp[:n] = lp
            posarr[:, (k - 1) * (CG // P):k * (CG // P)] = lpp.reshape(CG // P, P).T
        in_maps.append(dict(ctab=ctab, idx=idxp, slab=slab, w0=w0, w1=w1, ws=ws,
                            pos=posarr))
    return affine, in_maps


def kernel(**inputs) -> np.ndarray:
    from concourse.bass_utils import run_bass_kernel_spmd

    affine, in_maps = _prep(inputs)

    key = ("nc", affine)
    if key not in _CACHE:
        _CACHE[key] = _build_kernel(affine)
    nc = _CACHE[key]

    res = run_bass_kernel_spmd(nc, in_maps, list(range(NCORES)))

    per = NCH // NCORES
    blocks = [np.asarray(res.results[c]["out"])[:per] for c in range(NCORES)]
    return np.concatenate(blocks, axis=0).astype(np.float32)


# revision 4
# speedup vs baseline: 4.1821x; 1.6888x over previous
"""Trainium2 Bass kernel v5 for nn_CompositionalNetwork (ragged_sequence).

Per-token embedding concat (word[200] ++ tag[20]) -> per-chunk-length Linear
over chunks of 1..4 consecutive tokens -> scatter rows by pos.

Gather: InstDMAGatherAnt (vectorized Q7 descriptor generation) in
NON-transpose mode, round-robin over 4 SWDGE queues (multi-queue transpose
mode corrupts via the shared xbar; non-transpose is multi-queue-safe and
4 queues quadruple descriptor-generation throughput). dma_gather indices are
int16, so each core's 51200-token stream is split into two 25600-slot
windows with per-window host-compacted word tables (distinct rows in
first-use order -> near-sequential HBM access).

Compute: per 128-chunk tile, PE transposes flip [chunks, feat] into
[feat, chunks] (PSUM), DVE copies stage them to SBUF, then per position j
two matmuls contract word features, plus one matmul for the host-pretransposed
tag+bias slab:
  y[chunk,:] = sum_j xw_j[0:128] @ W0[j] + sum_j xw_j[128:200] @ W1[j]
             + slab[0:20k+1] @ Wslab_k

Sharding: data-parallel by output row range; per-core output is a contiguous
[20000, 200] block written with a strided affine DMA (row = 4*i + k-1), with
an indirect-scatter fallback if pos is not affine.
"""
import numpy as np
import ml_dtypes

bf16 = ml_dtypes.bfloat16

VOCAB = 128000
TAGS = 64
WD = 200
TD = 20
E = WD + TD
CD = 200
K = 4
C = 40000
S = 400000
NCH = K * C

NCORES = 8
P = 128
RW = 256            # padded compact-table row: 256 bf16 = 512 B
CG = 5120           # padded chunks per k-group per core
CPG = C // NCORES   # real chunks per group per core (5000)
OUTR = 4 * CG       # local out rows incl pad targets (20480)
NBT = 16            # max tiles per block
NBLK = NBT * P      # max chunks per block (2048)


def _blocks_for_group():
    """CG=5120 chunks per group: blocks of 2048, 2048, 1024."""
    blks = []
    off = 0
    while off < CG:
        n = min(NBLK, CG - off)
        blks.append((off, n))
        off += n
    return blks
NSLOT = CG * (1 + 2 + 3 + 4)  # 51200 gather slots per core
WSLOT = NSLOT // 2  # slots per index window (25600)
NCT = 2 * WSLOT     # compact table rows (2 windows of <=25600 distinct)
SLABP = 96          # slab partition dim (>= 20*K+1 = 81)
NQ = 4              # SWDGE queues

_CACHE = {}


def _build_kernel(affine):
    from concourse import bacc
    import concourse.tile as tile
    from concourse import mybir
    import concourse.bass as bass
    from concourse.bass import IndirectOffsetOnAxis
    from concourse.masks import make_identity

    nc = bacc.Bacc(None, num_swdge_queues=NQ, dynamic_dma_scratch_size=32768)

    ctab = nc.dram_tensor("ctab", [NCT, RW], mybir.dt.bfloat16, kind="ExternalInput")
    idx_d = nc.dram_tensor("idx", [P, NSLOT // 16], mybir.dt.int16, kind="ExternalInput")
    slab_d = nc.dram_tensor("slab", [SLABP, K * CG], mybir.dt.bfloat16, kind="ExternalInput")
    w0_d = nc.dram_tensor("w0", [10, P, CD], mybir.dt.bfloat16, kind="ExternalInput")
    w1_d = nc.dram_tensor("w1", [10, 72, CD], mybir.dt.bfloat16, kind="ExternalInput")
    ws_d = nc.dram_tensor("ws", [K, SLABP, CD], mybir.dt.bfloat16, kind="ExternalInput")
    pos_d = nc.dram_tensor("pos", [P, (CG // P) * K], mybir.dt.int32, kind="ExternalInput")
    out = nc.dram_tensor("out", [OUTR, CD], mybir.dt.float32, kind="ExternalOutput")

    with tile.TileContext(nc) as tc:
        with (
            tc.tile_pool(name="singles", bufs=1) as singles,
            tc.tile_pool(name="xp", bufs=8) as xp,
            tc.tile_pool(name="slp", bufs=3) as slp,
            tc.tile_pool(name="xtp", bufs=6) as xtp,
            tc.tile_pool(name="ysp", bufs=3) as ysp,
            tc.tile_pool(name="tpp", bufs=4, space="PSUM") as tpp,
            tc.tile_pool(name="ypp", bufs=4, space="PSUM") as ypp,
        ):
            ident = singles.tile([P, P], mybir.dt.bfloat16)
            make_identity(nc, ident[:])
            sidx = singles.tile([P, NSLOT // 16], mybir.dt.int16)
            nc.sync.dma_start(out=sidx[:], in_=idx_d[:])
            w0 = singles.tile([P, 10, CD], mybir.dt.bfloat16)
            nc.sync.dma_start(out=w0[:], in_=w0_d[:].rearrange("q f c -> f q c"))
            w1 = singles.tile([72, 10, CD], mybir.dt.bfloat16)
            nc.sync.dma_start(out=w1[:], in_=w1_d[:].rearrange("q f c -> f q c"))
            ws = singles.tile([SLABP, K, CD], mybir.dt.bfloat16)
            nc.sync.dma_start(out=ws[:], in_=ws_d[:].rearrange("k f c -> f k c"))
            spos = None
            if not affine:
                spos = singles.tile([P, (CG // P) * K], mybir.dt.int32)
                nc.sync.dma_start(out=spos[:], in_=pos_d[:])

            gq = [0]
            slot_base = 0
            for k in range(1, K + 1):
                q0 = (k - 1) * k // 2
                for (boff, bn) in _blocks_for_group():
                    nbt = bn // P
                    sl = slp.tile([SLABP, bn], mybir.dt.bfloat16, tag="sl")
                    c0 = (k - 1) * CG + boff
                    nc.sync.dma_start(out=sl[:], in_=slab_d[:, c0:c0 + bn])
                    xts = []
                    for j in range(k):
                        xt = xp.tile([P, nbt, RW], mybir.dt.bfloat16, tag="x")
                        s0 = slot_base + boff * k + j * bn
                        assert (s0 < WSLOT) == (s0 + bn <= WSLOT)
                        src = ctab[0:WSLOT] if s0 < WSLOT else ctab[WSLOT:NCT]
                        nc.gpsimd.dma_gather(
                            xt[:], src, sidx[:, s0 // 16:(s0 + bn) // 16],
                            bn, bn, RW, transpose=False,
                            queue_num=gq[0] % NQ, single_packet=False,
                        )
                        gq[0] += 1
                        xts.append(xt)
                    ystage = ysp.tile([P, nbt, CD], mybir.dt.float32)
                    for t in range(nbt):
                        y = ypp.tile([P, CD], mybir.dt.float32)
                        cs = t * P
                        for j in range(k):
                            tp = tpp.tile([P, 2 * P], mybir.dt.bfloat16)
                            nc.tensor.transpose(tp[0:P, 0:P], xts[j][:, t, 0:128], ident[:])
                            nc.tensor.transpose(tp[0:72, P:2 * P], xts[j][:, t, 128:200], ident[:])
                            xT = xtp.tile([P, 2 * P], mybir.dt.bfloat16, tag="xT")
                            nc.vector.tensor_copy(xT[:, 0:P], tp[:, 0:P])
                            nc.vector.tensor_copy(xT[0:72, P:2 * P], tp[0:72, P:2 * P])
                            nc.tensor.matmul(
                                y[:], lhsT=xT[:, 0:P], rhs=w0[:, q0 + j, :],
                                start=(j == 0), stop=False,
                            )
                            nc.tensor.matmul(
                                y[:], lhsT=xT[0:72, P:2 * P], rhs=w1[0:72, q0 + j, :],
                                start=False, stop=False,
                            )
                        nc.tensor.matmul(
                            y[:], lhsT=sl[0:20 * k + 1, cs:cs + P],
                            rhs=ws[0:20 * k + 1, k - 1, :],
                            start=False, stop=True,
                        )
                        nc.vector.tensor_copy(ystage[:, t, :], y[:])
                    if affine:
                        # out row = 4*(boff + t*128 + p) + (k-1)
                        dst = bass.AP(
                            tensor=out[:].tensor,
                            offset=(4 * boff + (k - 1)) * CD,
                            ap=[[4 * CD, P], [4 * P * CD, nbt], [1, CD]],
                        )
                        nc.sync.dma_start(out=dst, in_=ystage[:, :, :])
                    else:
                        for t in range(nbt):
                            tt = boff // P + t
                            nc.gpsimd.indirect_dma_start(
                                out=out[:],
                                out_offset=IndirectOffsetOnAxis(
                                    ap=spos[:, (k - 1) * (CG // P) + tt:(k - 1) * (CG // P) + tt + 1],
                                    axis=0,
                                ),
                                in_=ystage[:, t, :],
                                in_offset=None,
                            )
                slot_base += k * CG
    nc.compile()
    return nc


def _prep(inputs):
    """Host-side shard + pack. Returns (affine, in_maps)."""
    tok = np.asarray(inputs["token_indices"]).astype(np.int64)
    tag = np.asarray(inputs["tag_indices"]).astype(np.int64)
    word_table = np.asarray(inputs["word_table"], dtype=np.float32)
    tag_table = np.asarray(inputs["tag_table"], dtype=np.float32)

    wtab_bf = word_table.astype(bf16)             # [V, 200]
    tagemb = tag_table.astype(bf16)               # [TAGS, 20]

    # packed weights (shared by all cores)
    w0 = np.zeros((10, P, CD), dtype=np.float32)
    w1 = np.zeros((10, 72, CD), dtype=np.float32)
    ws = np.zeros((K, SLABP, CD), dtype=np.float32)
    for k in range(1, K + 1):
        Wk = np.asarray(inputs[f"W{k}"], dtype=np.float32)
        bk = np.asarray(inputs[f"b{k}"], dtype=np.float32)
        q0 = (k - 1) * k // 2
        for j in range(k):
            off = j * E
            w0[q0 + j] = Wk[:, off:off + 128].T
            w1[q0 + j] = Wk[:, off + 128:off + 200].T
            ws[k - 1, 20 * j:20 * j + 20] = Wk[:, off + 200:off + 220].T
        ws[k - 1, 20 * k] = bk
    w0 = w0.astype(bf16)
    w1 = w1.astype(bf16)
    ws = ws.astype(bf16)

    affine = True
    shards = []  # per core: dict k -> (chunk_ids[CG], local_pos[CG], n)
    for c in range(NCORES):
        lo, hi = c * (NCH // NCORES), (c + 1) * (NCH // NCORES)
        per_k = {}
        for k in range(1, K + 1):
            pos = np.asarray(inputs[f"pos{k}"]).astype(np.int64)
            sel = np.nonzero((pos >= lo) & (pos < hi))[0]
            lp = pos[sel] - lo
            order = np.argsort(lp, kind="stable")
            sel = sel[order]
            lp = lp[order]
            n = len(sel)
            if n > CG:
                raise ValueError("shard overflow; unbalanced pos distribution")
            if n != CPG or not np.array_equal(lp, 4 * np.arange(n) + (k - 1)):
                affine = False
            per_k[k] = (sel, lp, n)
        shards.append(per_k)

    in_maps = []
    for c in range(NCORES):
        # ---- gather slot stream: vocab id per slot, in op order ----
        vids = np.zeros(NSLOT, dtype=np.int64)
        slot_base = 0
        for k in range(1, K + 1):
            starts = np.asarray(inputs[f"starts{k}"]).astype(np.int64)
            sel, lp, n = shards[c][k]
            st = np.zeros(CG, dtype=np.int64)
            st[:n] = starts[sel]
            for (boff, bn) in _blocks_for_group():
                for j in range(k):
                    s0 = slot_base + boff * k + j * bn
                    cc = st[boff:boff + bn] + j
                    vids[s0:s0 + bn] = tok[np.clip(cc, 0, S - 1)]
            slot_base += k * CG

        # ---- per-window dedup -> compact table + int16 indices ----
        idx16 = np.zeros(NSLOT, dtype=np.int16)
        ctab = np.zeros((NCT, RW), dtype=bf16)
        for w in range(2):
            sl = slice(w * WSLOT, (w + 1) * WSLOT)
            uniq, inv = np.unique(vids[sl], return_inverse=True)
            first_use = np.full(len(uniq), NSLOT, dtype=np.int64)
            np.minimum.at(first_use, inv, np.arange(WSLOT))
            order = np.argsort(first_use, kind="stable")
            rank = np.empty_like(order)
            rank[order] = np.arange(len(uniq))
            nw = len(uniq)
            assert nw <= WSLOT
            idx16[sl] = rank[inv].astype(np.int16)
            ctab[w * WSLOT:w * WSLOT + nw, 0:WD] = wtab_bf[uniq[order]]

        # idx packed layout: slot i -> [16*g + i%16, i//16] replicated g=0..7
        idxp = np.tile(idx16.reshape(NSLOT // 16, 16).T, (8, 1))

        # ---- slab: [96, K*CG] tag embeddings + ones row ----
        slab = np.zeros((SLABP, K * CG), dtype=bf16)
        posarr = np.zeros((P, (CG // P) * K), dtype=np.int32)
        for k in range(1, K + 1):
            starts = np.asarray(inputs[f"starts{k}"]).astype(np.int64)
            sel, lp, n = shards[c][k]
            st = np.zeros(CG, dtype=np.int64)
            st[:n] = starts[sel]
            col0 = (k - 1) * CG
            for j in range(k):
                tg = tag[np.clip(st + j, 0, S - 1)]
                tv = tagemb[tg]                     # [CG, 20]
                tv[n:] = 0
                slab[20 * j:20 * j + 20, col0:col0 + CG] = tv.T
            onesr = np.zeros(CG, dtype=bf16)
            onesr[:n] = 1.0
            slab[20 * k, col0:col0 + CG] = onesr
            lpp = np.full(CG, OUTR - P, dtype=np.int64)
            lpp[:n] = lp
            posarr[:, (k - 1) * (CG // P):k * (CG // P)] = lpp.reshape(CG // P, P).T
        in_maps.append(dict(ctab=ctab, idx=idxp, slab=slab, w0=w0, w1=w1, ws=ws,
                            pos=posarr))
    return affine, in_maps


def kernel(**inputs) -> np.ndarray:
    from concourse.bass_utils import run_bass_kernel_spmd

    affine, in_maps = _prep(inputs)

    key = ("nc", affine)
    if key not in _CACHE:
        _CACHE[key] = _build_kernel(affine)
    nc = _CACHE[key]

    res = run_bass_kernel_spmd(nc, in_maps, list(range(NCORES)))

    per = NCH // NCORES
    blocks = [np.asarray(res.results[c]["out"])[:per] for c in range(NCORES)]
    return np.concatenate(blocks, axis=0).astype(np.float32)
